# revision 8
# baseline (speedup 1.0000x reference)
"""nn_ClusterAssignment (vq_codebook) Trainium2 kernel.

kernel(batch, cluster_centers) -> (out, lossp, lossd)
  batch           [64, 4096, 128] fp32
  cluster_centers [64, 128]       fp32
  out             [64, 4096, 64]  fp32 = entmax15(soft_assign, axis=2)
  lossp, lossd    fp32 scalars (depend only on cluster_centers)

Sharding: data-parallel over B across 8 NeuronCores (8 B-slices each,
32768 rows/core). cluster_centers replicated (pre-scaled on host by
1/sqrt(||c||) so the device computes soft_assign as a plain square).
The tiny lossp/lossd terms (~1 MFLOP, centers only) are computed on
host in fp32 numpy, mirroring the reference op-for-op.

Device math per row (K=64): x = dot^2/2; entmax tau via Newton on
g(t) = 0.5*sum(relu(x-t)^2) - 0.5 starting at t0 = max(x)-1 (g convex
and decreasing, so iterates approach the root from below); p =
relu(x - tau)^2.  The state kept is nt = -t so the scalar engine can
evaluate relu(x + nt) as an activation bias.  4 iterations reach
~1e-5 of the fp32 floor; no sort needed.

Engine split (measured: DVE elementwise-max runs at ~half rate, so the
relu goes to the scalar engine for most tiles):
  PE:      ldweights(batchT tile) + matmul(rhs=centersT) -> psum
  ScalarE: x = Square(psum/sqrt2); y = Relu(x + nt) (+accum s1) for
           SCALAR_RELU_TILES of 8; final Square
  VectorE: max via tensor_scalar reduce-accum; remaining relus via
           scalar_tensor_tensor; y^2 (+accum s2) via same-tile STT;
           batched [128,8] state updates.
"""

import math
import os
import sys

import numpy as np

for _p in ("/opt/trn_rl_repo", "/root/.axon_site/_ro/trn_rl_repo"):
    if _p not in sys.path and os.path.isdir(_p):
        sys.path.append(_p)

import concourse.bass as bass  # noqa: E402
import concourse.tile as tile  # noqa: E402
from concourse import bacc, mybir  # noqa: E402
from concourse.bass_utils import run_bass_kernel_spmd  # noqa: E402

F32 = mybir.dt.float32
AX = mybir.AxisListType
OP = mybir.AluOpType
AF = mybir.ActivationFunctionType

B, N, D, K = 64, 4096, 128, 64
NCORES = 8
R_FULL = B * N // NCORES  # rows per core

NEWTON_ITERS = 4
SCALAR_RELU_TILES = 8   # of 8: Newton relu on scalar engine
FINAL_SCALAR_TILES = 8  # of 8: final relu on scalar engine too

# test.py hooks
TRACE = False
TRACE_KWARGS = {}
LAST_RESULTS = None

_CACHE = {}


def _build(R=R_FULL, newton_iters=NEWTON_ITERS,
           scalar_relu_tiles=SCALAR_RELU_TILES,
           final_scalar_tiles=FINAL_SCALAR_TILES,
           num_devices=NCORES):
    G = 8                 # 128-row tiles per group
    GROUP_ROWS = G * 128  # 1024
    assert R % GROUP_ROWS == 0
    ngroups = R // GROUP_ROWS
    inv_sqrt2 = 1.0 / math.sqrt(2.0)

    nc = bacc.Bacc("TRN2", target_bir_lowering=False, debug=False,
                   num_devices=num_devices)
    bt_d = nc.dram_tensor("batchT", [D, R], F32, kind="ExternalInput")
    ct_d = nc.dram_tensor("centersT", [D, K], F32, kind="ExternalInput")
    out_d = nc.dram_tensor("out", [R, K], F32, kind="ExternalOutput")

    with tile.TileContext(nc) as tc:
        with (
            tc.tile_pool(name="const", bufs=1) as const_pool,
            tc.tile_pool(name="bt", bufs=3) as bt_pool,
            tc.tile_pool(name="x", bufs=3, space="PSUM") as x_pool,
            tc.tile_pool(name="y", bufs=3) as y_pool,
            tc.tile_pool(name="o", bufs=3) as o_pool,
            tc.tile_pool(name="st", bufs=4) as st_pool,
            tc.tile_pool(name="ps", bufs=3, space="PSUM") as ps_pool,
        ):
            ct = const_pool.tile([D, K], F32)
            nc.sync.dma_start(out=ct, in_=ct_d[:, :])
            zeros = const_pool.tile([128, K], F32)
            nc.vector.memset(zeros, 0.0)

            for g in range(ngroups):
                base = g * GROUP_ROWS
                bt = bt_pool.tile([D, GROUP_ROWS], F32)
                nc.sync.dma_start(out=bt, in_=bt_d[:, base:base + GROUP_ROWS])
                # column c of bt holds row base+c; tile j takes columns
                # p*8+j so psum partition p holds row base+p*8+j, making
                # the group's output DMA one contiguous 256KB write.
                bt_r = bt.rearrange("d (p j) -> d j p", j=G)

                x = x_pool.tile([128, G * K], F32)
                y = y_pool.tile([128, G * K], F32)
                o = o_pool.tile([128, G * K], F32)
                m = st_pool.tile([128, G], F32)
                nt = st_pool.tile([128, G], F32)
                s1 = st_pool.tile([128, G], F32)
                s2 = st_pool.tile([128, G], F32)
                rs1 = st_pool.tile([128, G], F32)
                dlt = st_pool.tile([128, G], F32)

                # 8 matmuls land in slices of ONE psum bank
                ps = ps_pool.tile([128, G * K], F32)
                for j in range(G):
                    nc.tensor.matmul(ps[:, j * K:(j + 1) * K],
                                     bt_r[:, j, :], ct, start=True,
                                     stop=True)
                # x = (dot/sqrt(2))^2 = dot^2/2, whole group in one op
                nc.scalar.activation(out=x, in_=ps, func=AF.Square,
                                     scale=inv_sqrt2)
                x3 = x.rearrange("p (g k) -> p g k", g=G)
                y3 = y.rearrange("p (g k) -> p g k", g=G)
                # m = per-tile max, one grouped reduce
                nc.vector.reduce_max(m, x3, axis=AX.X)
                # nt0 = 1 - m   (nt = -t)
                nc.vector.tensor_scalar(out=nt, in0=m, scalar1=-1.0,
                                        scalar2=1.0, op0=OP.mult, op1=OP.add)

                for it in range(newton_iters):
                    for j in range(G):
                        xs = x[:, j * K:(j + 1) * K]
                        ys = y[:, j * K:(j + 1) * K]
                        if j < scalar_relu_tiles:
                            # y = relu(x + nt)                [ScalarE]
                            nc.scalar.activation(
                                out=ys, in_=xs, func=AF.Relu,
                                bias=nt[:, j:j + 1], scale=1.0)
                        else:
                            # y = max(x + nt, 0)              [VectorE]
                            nc.vector.scalar_tensor_tensor(
                                out=ys, in0=xs, scalar=nt[:, j:j + 1],
                                in1=zeros, op0=OP.add, op1=OP.max)
                    # s1 = per-tile sum(y), grouped reduce
                    nc.vector.reduce_sum(s1, y3, axis=AX.X)
                    # y <- y^2 group-wide
                    nc.vector.tensor_mul(out=y, in0=y, in1=y)
                    # s2 = per-tile sum(y^2), grouped reduce
                    nc.vector.reduce_sum(s2, y3, axis=AX.X)
                    # nt -= (0.5*s2 - 0.5)/s1
                    nc.vector.reciprocal(out=rs1, in_=s1)
                    # dlt2 = (s2 - 1) * rs1  (= 2*delta)
                    nc.vector.scalar_tensor_tensor(
                        out=dlt, in0=s2, scalar=1.0, in1=rs1,
                        op0=OP.subtract, op1=OP.mult)
                    # nt = (dlt2 * -0.5) + nt
                    nc.vector.scalar_tensor_tensor(
                        out=nt, in0=dlt, scalar=-0.5, in1=nt,
                        op0=OP.mult, op1=OP.add)

                for j in range(G):
                    xs = x[:, j * K:(j + 1) * K]
                    ys = y[:, j * K:(j + 1) * K]
                    if j < final_scalar_tiles:
                        nc.scalar.activation(out=ys, in_=xs, func=AF.Relu,
                                             bias=nt[:, j:j + 1], scale=1.0)
                    else:
                        # y = max(x + nt, 0)
                        nc.vector.tensor_scalar(
                            out=ys, in0=xs,
                            scalar1=nt[:, j:j + 1], scalar2=0.0,
                            op0=OP.add, op1=OP.max)
                # p = y^2 group-wide into the DMA staging tile [ScalarE]
                nc.scalar.activation(out=o, in_=y, func=AF.Square)

                out_ap = out_d[base:base + GROUP_ROWS, :].rearrange(
                    "(p j) k -> p (j k)", j=G)
                nc.sync.dma_start(out=out_ap, in_=o)

    nc.compile()
    return nc


def _get_nc():
    key = (R_FULL, NEWTON_ITERS, SCALAR_RELU_TILES, FINAL_SCALAR_TILES)
    if key not in _CACHE:
        _CACHE[key] = _build(*key)
    return _CACHE[key]


def _entmax15_np(x):
    """Exact fp32 numpy port of the reference's sort-based entmax15
    along the last axis."""
    x = x.astype(np.float32) / np.float32(2.0)
    x = x - x.max(-1, keepdims=True)
    xs = np.flip(np.sort(x, axis=-1), axis=-1)
    k = x.shape[-1]
    rho = np.arange(1, k + 1, dtype=np.float32)
    mean = (np.cumsum(xs, -1, dtype=np.float32) / rho).astype(np.float32)
    mean_sq = (np.cumsum(xs * xs, -1, dtype=np.float32) / rho).astype(
        np.float32)
    ss = rho * (mean_sq - mean * mean)
    delta = (np.float32(1.0) - ss) / rho
    tau = mean - np.sqrt(np.clip(delta, 0.0, None))
    support = (tau <= xs).sum(-1, keepdims=True)
    tau_star = np.take_along_axis(tau, support - 1, axis=-1)
    return np.square(np.clip(x - tau_star, 0.0, None)).astype(np.float32)


def _losses_np(centers):
    c = centers.astype(np.float32)
    k = c.shape[0]
    p = _entmax15_np(c)
    m = (p @ p.T - np.eye(k, dtype=np.float32)).astype(np.float32)
    lossp = np.float32(np.linalg.norm((m @ m).astype(np.float32)) / k)
    eps = np.float32(1e-6)
    diffs = (c[:, None, :] - c[None, :, :] + eps).astype(np.float32)
    dist = np.sqrt((diffs * diffs).sum(-1, dtype=np.float32))
    upper = np.triu(np.ones((k, k), np.float32), k=1)
    total = np.float32(1e-10) + (dist * upper).sum(dtype=np.float32)
    n_pairs = k * (k - 1) // 2
    lossd = np.float32(1.0 / total / n_pairs)
    return lossp, lossd


def kernel(batch, cluster_centers):
    global LAST_RESULTS
    batch = np.ascontiguousarray(np.asarray(batch, dtype=np.float32))
    centers = np.ascontiguousarray(
        np.asarray(cluster_centers, dtype=np.float32))
    assert batch.shape == (B, N, D) and centers.shape == (K, D)

    # fold 1/sqrt(||c||) into the centers: (dot/sqrt(norm))^2 = dot^2/norm
    norm = np.sqrt((centers * centers).sum(-1, dtype=np.float32))
    cs = (centers / np.sqrt(norm)[:, None]).astype(np.float32)
    csT = np.ascontiguousarray(cs.T)  # [D, K]

    bpc = B // NCORES
    in_maps = []
    for i in range(NCORES):
        shard = batch[i * bpc:(i + 1) * bpc].reshape(R_FULL, D)
        in_maps.append({
            "batchT": np.ascontiguousarray(shard.T),
            "centersT": csT,
        })

    nc = _get_nc()
    res = run_bass_kernel_spmd(nc, in_maps, core_ids=list(range(NCORES)),
                               trace=TRACE, **TRACE_KWARGS)
    LAST_RESULTS = res

    out = np.concatenate([np.asarray(r["out"]) for r in res.results],
                         axis=0).reshape(B, N, K)
    lossp, lossd = _losses_np(centers)
    return out, lossp, lossd


# revision 10
# speedup vs baseline: 1.3795x; 1.3795x over previous
"""nn_ClusterAssignment (vq_codebook) Trainium2 kernel.

kernel(batch, cluster_centers) -> (out, lossp, lossd)
  batch           [64, 4096, 128] fp32
  cluster_centers [64, 128]       fp32
  out             [64, 4096, 64]  fp32 = entmax15(soft_assign, axis=2)
  lossp, lossd    fp32 scalars (depend only on cluster_centers)

Sharding: data-parallel over B across 8 NeuronCores (8 B-slices each,
32768 rows/core). cluster_centers replicated (pre-scaled on host by
1/sqrt(||c||) so the device computes soft_assign as a plain square).
The tiny lossp/lossd terms (~1 MFLOP, centers only) are computed on
host in fp32 numpy, mirroring the reference op-for-op.

Device math per row (K=64): x = dot^2/2; entmax tau via Newton on
g(t) = 0.5*sum(relu(x-t)^2) - 0.5 starting at t0 = max(x)-1 (g convex
and decreasing, so iterates approach the root from below); p =
relu(x - tau)^2.  The state kept is nt = -t so the scalar engine can
evaluate relu(x + nt) as an activation bias.  4 iterations reach
~1e-5 of the fp32 floor; no sort needed.

Engine split (measured: DVE elementwise-max runs at ~half rate, so the
relu goes to the scalar engine for most tiles):
  PE:      ldweights(batchT tile) + matmul(rhs=centersT) -> psum
  ScalarE: x = Square(psum/sqrt2); y = Relu(x + nt) (+accum s1) for
           SCALAR_RELU_TILES of 8; final Square
  VectorE: max via tensor_scalar reduce-accum; remaining relus via
           scalar_tensor_tensor; y^2 (+accum s2) via same-tile STT;
           batched [128,8] state updates.
"""

import math
import os
import sys

import numpy as np

for _p in ("/opt/trn_rl_repo", "/root/.axon_site/_ro/trn_rl_repo"):
    if _p not in sys.path and os.path.isdir(_p):
        sys.path.append(_p)

import concourse.bass as bass  # noqa: E402
import concourse.tile as tile  # noqa: E402
from concourse import bacc, mybir  # noqa: E402
from concourse.bass_utils import run_bass_kernel_spmd  # noqa: E402

F32 = mybir.dt.float32
AX = mybir.AxisListType
OP = mybir.AluOpType
AF = mybir.ActivationFunctionType

B, N, D, K = 64, 4096, 128, 64
NCORES = 8
R_FULL = B * N // NCORES  # rows per core

NEWTON_ITERS = 4
SCALAR_RELU_TILES = 8   # of 8: Newton relu on scalar engine
FINAL_SCALAR_TILES = 8  # of 8: final relu on scalar engine too

# test.py hooks
TRACE = False
TRACE_KWARGS = {}
LAST_RESULTS = None

_CACHE = {}


def _build(R=R_FULL, newton_iters=NEWTON_ITERS,
           scalar_relu_tiles=SCALAR_RELU_TILES,
           final_scalar_tiles=FINAL_SCALAR_TILES,
           num_devices=NCORES):
    G = 8                 # 128-row tiles per group
    GROUP_ROWS = G * 128  # 1024
    assert R % GROUP_ROWS == 0
    ngroups = R // GROUP_ROWS
    inv_sqrt2 = 1.0 / math.sqrt(2.0)

    nc = bacc.Bacc("TRN2", target_bir_lowering=False, debug=False,
                   num_devices=num_devices)
    bt_d = nc.dram_tensor("batchT", [D, R], F32, kind="ExternalInput")
    ct_d = nc.dram_tensor("centersT", [D, K], F32, kind="ExternalInput")
    out_d = nc.dram_tensor("out", [R, K], F32, kind="ExternalOutput")

    with tile.TileContext(nc) as tc:
        with (
            tc.tile_pool(name="const", bufs=1) as const_pool,
            tc.tile_pool(name="bt", bufs=3) as bt_pool,
            tc.tile_pool(name="x", bufs=3) as x_pool,
            tc.tile_pool(name="y", bufs=3) as y_pool,
            tc.tile_pool(name="o", bufs=3) as o_pool,
            tc.tile_pool(name="st", bufs=4) as st_pool,
            tc.tile_pool(name="ps", bufs=4, space="PSUM") as ps_pool,
        ):
            ct = const_pool.tile([D, K], F32)
            nc.sync.dma_start(out=ct, in_=ct_d[:, :])
            zeros = const_pool.tile([128, K], F32)
            nc.vector.memset(zeros, 0.0)

            for g in range(ngroups):
                base = g * GROUP_ROWS
                bt = bt_pool.tile([D, GROUP_ROWS], F32)
                nc.sync.dma_start(out=bt, in_=bt_d[:, base:base + GROUP_ROWS])
                # column c of bt holds row base+c; tile j takes columns
                # p*8+j so psum partition p holds row base+p*8+j, making
                # the group's output DMA one contiguous 256KB write.
                bt_r = bt.rearrange("d (p j) -> d j p", j=G)

                x = x_pool.tile([128, G * K], F32)
                y = y_pool.tile([128, G * K], F32)
                o = o_pool.tile([128, G * K], F32)
                m = st_pool.tile([128, G], F32)
                nt = st_pool.tile([128, G], F32)
                s1 = st_pool.tile([128, G], F32)
                s2 = st_pool.tile([128, G], F32)
                rs1 = st_pool.tile([128, G], F32)
                dlt = st_pool.tile([128, G], F32)

                # 8 matmuls land in slices of ONE psum bank
                ps = ps_pool.tile([128, G * K], F32)
                for j in range(G):
                    nc.tensor.matmul(ps[:, j * K:(j + 1) * K],
                                     bt_r[:, j, :], ct, start=True,
                                     stop=True)
                # x = (dot/sqrt(2))^2 = dot^2/2, whole group in one op
                nc.scalar.activation(out=x, in_=ps, func=AF.Square,
                                     scale=inv_sqrt2)
                x3 = x.rearrange("p (g k) -> p g k", g=G)
                y3 = y.rearrange("p (g k) -> p g k", g=G)
                # m = per-tile max, one grouped reduce
                nc.vector.reduce_max(m, x3, axis=AX.X)
                # nt0 = 1 - m   (nt = -t)
                nc.vector.tensor_scalar(out=nt, in0=m, scalar1=-1.0,
                                        scalar2=1.0, op0=OP.mult, op1=OP.add)

                # nt broadcast over k via a stride-0 middle AP dim
                nt_b = bass.AP(tensor=nt.tensor, offset=nt.offset,
                               ap=[nt.ap[0], nt.ap[1], [0, K]])

                for it in range(newton_iters):
                    # z = x + nt (group-wide broadcast add)   [VectorE]
                    nc.vector.tensor_add(out=y3, in0=x3, in1=nt_b)
                    # y = relu(z) group-wide                  [ScalarE]
                    nc.scalar.activation(out=y, in_=y, func=AF.Relu)
                    # s1 = per-tile sum(y), grouped reduce    [VectorE]
                    nc.vector.reduce_sum(s1, y3, axis=AX.X)
                    # y <- y^2 group-wide                     [ScalarE]
                    nc.scalar.activation(out=y, in_=y, func=AF.Square)
                    # s2 = per-tile sum(y^2), grouped reduce  [VectorE]
                    nc.vector.reduce_sum(s2, y3, axis=AX.X)
                    # nt -= (0.5*s2 - 0.5)/s1
                    nc.vector.reciprocal(out=rs1, in_=s1)
                    # dlt2 = (s2 - 1) * rs1  (= 2*delta)
                    nc.vector.scalar_tensor_tensor(
                        out=dlt, in0=s2, scalar=1.0, in1=rs1,
                        op0=OP.subtract, op1=OP.mult)
                    # nt = (dlt2 * -0.5) + nt
                    nc.vector.scalar_tensor_tensor(
                        out=nt, in0=dlt, scalar=-0.5, in1=nt,
                        op0=OP.mult, op1=OP.add)

                # final: p = relu(x + nt)^2
                nc.vector.tensor_add(out=y3, in0=x3, in1=nt_b)
                nc.scalar.activation(out=y, in_=y, func=AF.Relu)
                # p into the DMA staging tile                 [ScalarE]
                nc.scalar.activation(out=o, in_=y, func=AF.Square)

                out_ap = out_d[base:base + GROUP_ROWS, :].rearrange(
                    "(p j) k -> p (j k)", j=G)
                nc.sync.dma_start(out=out_ap, in_=o)

    nc.compile()
    return nc


def _get_nc():
    key = (R_FULL, NEWTON_ITERS, SCALAR_RELU_TILES, FINAL_SCALAR_TILES)
    if key not in _CACHE:
        _CACHE[key] = _build(*key)
    return _CACHE[key]


def _entmax15_np(x):
    """Exact fp32 numpy port of the reference's sort-based entmax15
    along the last axis."""
    x = x.astype(np.float32) / np.float32(2.0)
    x = x - x.max(-1, keepdims=True)
    xs = np.flip(np.sort(x, axis=-1), axis=-1)
    k = x.shape[-1]
    rho = np.arange(1, k + 1, dtype=np.float32)
    mean = (np.cumsum(xs, -1, dtype=np.float32) / rho).astype(np.float32)
    mean_sq = (np.cumsum(xs * xs, -1, dtype=np.float32) / rho).astype(
        np.float32)
    ss = rho * (mean_sq - mean * mean)
    delta = (np.float32(1.0) - ss) / rho
    tau = mean - np.sqrt(np.clip(delta, 0.0, None))
    support = (tau <= xs).sum(-1, keepdims=True)
    tau_star = np.take_along_axis(tau, support - 1, axis=-1)
    return np.square(np.clip(x - tau_star, 0.0, None)).astype(np.float32)


def _losses_np(centers):
    c = centers.astype(np.float32)
    k = c.shape[0]
    p = _entmax15_np(c)
    m = (p @ p.T - np.eye(k, dtype=np.float32)).astype(np.float32)
    lossp = np.float32(np.linalg.norm((m @ m).astype(np.float32)) / k)
    eps = np.float32(1e-6)
    diffs = (c[:, None, :] - c[None, :, :] + eps).astype(np.float32)
    dist = np.sqrt((diffs * diffs).sum(-1, dtype=np.float32))
    upper = np.triu(np.ones((k, k), np.float32), k=1)
    total = np.float32(1e-10) + (dist * upper).sum(dtype=np.float32)
    n_pairs = k * (k - 1) // 2
    lossd = np.float32(1.0 / total / n_pairs)
    return lossp, lossd


def kernel(batch, cluster_centers):
    global LAST_RESULTS
    batch = np.ascontiguousarray(np.asarray(batch, dtype=np.float32))
    centers = np.ascontiguousarray(
        np.asarray(cluster_centers, dtype=np.float32))
    assert batch.shape == (B, N, D) and centers.shape == (K, D)

    # fold 1/sqrt(||c||) into the centers: (dot/sqrt(norm))^2 = dot^2/norm
    norm = np.sqrt((centers * centers).sum(-1, dtype=np.float32))
    cs = (centers / np.sqrt(norm)[:, None]).astype(np.float32)
    csT = np.ascontiguousarray(cs.T)  # [D, K]

    bpc = B // NCORES
    in_maps = []
    for i in range(NCORES):
        shard = batch[i * bpc:(i + 1) * bpc].reshape(R_FULL, D)
        in_maps.append({
            "batchT": np.ascontiguousarray(shard.T),
            "centersT": csT,
        })

    nc = _get_nc()
    res = run_bass_kernel_spmd(nc, in_maps, core_ids=list(range(NCORES)),
                               trace=TRACE, **TRACE_KWARGS)
    LAST_RESULTS = res

    out = np.concatenate([np.asarray(r["out"]) for r in res.results],
                         axis=0).reshape(B, N, K)
    lossp, lossd = _losses_np(centers)
    return out, lossp, lossd


# revision 12
# speedup vs baseline: 1.4393x; 1.0433x over previous
"""nn_ClusterAssignment (vq_codebook) Trainium2 kernel.

kernel(batch, cluster_centers) -> (out, lossp, lossd)
  batch           [64, 4096, 128] fp32
  cluster_centers [64, 128]       fp32
  out             [64, 4096, 64]  fp32 = entmax15(soft_assign, axis=2)
  lossp, lossd    fp32 scalars (depend only on cluster_centers)

Sharding: data-parallel over B across 8 NeuronCores (8 B-slices each,
32768 rows/core). cluster_centers replicated (pre-scaled on host by
1/sqrt(||c||) so the device computes soft_assign as a plain square).
The tiny lossp/lossd terms (~1 MFLOP, centers only) are computed on
host in fp32 numpy, mirroring the reference op-for-op.

Device math per row (K=64): x = dot^2/2; entmax tau via Newton on
g(t) = 0.5*sum(relu(x-t)^2) - 0.5 starting at t0 = max(x)-1 (g convex
and decreasing, so iterates approach the root from below); p =
relu(x - tau)^2.  The state kept is nt = -t so the scalar engine can
evaluate relu(x + nt) as an activation bias.  4 iterations reach
~1e-5 of the fp32 floor; no sort needed.

Engine split (measured: DVE elementwise-max runs at ~half rate, so the
relu goes to the scalar engine for most tiles):
  PE:      ldweights(batchT tile) + matmul(rhs=centersT) -> psum
  ScalarE: x = Square(psum/sqrt2); y = Relu(x + nt) (+accum s1) for
           SCALAR_RELU_TILES of 8; final Square
  VectorE: max via tensor_scalar reduce-accum; remaining relus via
           scalar_tensor_tensor; y^2 (+accum s2) via same-tile STT;
           batched [128,8] state updates.
"""

import math
import os
import sys

import numpy as np

for _p in ("/opt/trn_rl_repo", "/root/.axon_site/_ro/trn_rl_repo"):
    if _p not in sys.path and os.path.isdir(_p):
        sys.path.append(_p)

import concourse.bass as bass  # noqa: E402
import concourse.tile as tile  # noqa: E402
from concourse import bacc, mybir  # noqa: E402
from concourse.bass_utils import run_bass_kernel_spmd  # noqa: E402

F32 = mybir.dt.float32
AX = mybir.AxisListType
OP = mybir.AluOpType
AF = mybir.ActivationFunctionType

B, N, D, K = 64, 4096, 128, 64
NCORES = 8
R_FULL = B * N // NCORES  # rows per core

NEWTON_ITERS = 4
SCALAR_RELU_TILES = 8   # of 8: Newton relu on scalar engine
FINAL_SCALAR_TILES = 8  # of 8: final relu on scalar engine too

# test.py hooks
TRACE = False
TRACE_KWARGS = {}
LAST_RESULTS = None

_CACHE = {}


def _build(R=R_FULL, newton_iters=NEWTON_ITERS,
           scalar_relu_tiles=SCALAR_RELU_TILES,
           final_scalar_tiles=FINAL_SCALAR_TILES,
           num_devices=NCORES):
    G = 8                 # 128-row tiles per group
    GROUP_ROWS = G * 128  # 1024
    assert R % GROUP_ROWS == 0
    ngroups = R // GROUP_ROWS
    inv_sqrt2 = 1.0 / math.sqrt(2.0)

    nc = bacc.Bacc("TRN2", target_bir_lowering=False, debug=False,
                   num_devices=num_devices)
    bt_d = nc.dram_tensor("batchT", [D, R], F32, kind="ExternalInput")
    ct_d = nc.dram_tensor("centersT", [D, K], F32, kind="ExternalInput")
    out_d = nc.dram_tensor("out", [R, K], F32, kind="ExternalOutput")

    with tile.TileContext(nc) as tc:
        with (
            tc.tile_pool(name="const", bufs=1) as const_pool,
            tc.tile_pool(name="bt", bufs=2) as bt_pool,
            tc.tile_pool(name="x", bufs=2) as x_pool,
            tc.tile_pool(name="y", bufs=2) as y_pool,
            tc.tile_pool(name="o", bufs=2) as o_pool,
            tc.tile_pool(name="st", bufs=2) as st_pool,
            tc.tile_pool(name="ps", bufs=2, space="PSUM") as ps_pool,
        ):
            ct = const_pool.tile([D, K], F32)
            nc.sync.dma_start(out=ct, in_=ct_d[:, :])
            zeros = const_pool.tile([128, K], F32)
            nc.vector.memset(zeros, 0.0)

            def emit_head(g):
                """DMA + matmuls + x/max/nt0 for group g; returns state."""
                base = g * GROUP_ROWS
                bt = bt_pool.tile([D, GROUP_ROWS], F32, name=f"bt{g % 2}")
                nc.sync.dma_start(out=bt, in_=bt_d[:, base:base + GROUP_ROWS])
                # column c of bt holds row base+c; tile j takes columns
                # p*8+j so psum partition p holds row base+p*8+j, making
                # the group's output DMA one contiguous 256KB write.
                bt_r = bt.rearrange("d (p j) -> d j p", j=G)

                x = x_pool.tile([128, G * K], F32, name=f"x{g % 2}")
                # y holds [relu(z) | relu(z)^2] halves for one merged reduce
                y = y_pool.tile([128, 2 * G * K], F32, name=f"y{g % 2}")
                o = o_pool.tile([128, G * K], F32, name=f"o{g % 2}")
                m = st_pool.tile([128, G], F32, name=f"m{g % 2}")
                nt = st_pool.tile([128, G], F32, name=f"nt{g % 2}")
                s12 = st_pool.tile([128, 2 * G], F32, name=f"s12_{g % 2}")
                rs1 = st_pool.tile([128, G], F32, name=f"rs1_{g % 2}")
                dlt = st_pool.tile([128, G], F32, name=f"dlt{g % 2}")

                # 8 matmuls land in slices of ONE psum bank
                ps = ps_pool.tile([128, G * K], F32, name=f"ps{g % 2}")
                for j in range(G):
                    nc.tensor.matmul(ps[:, j * K:(j + 1) * K],
                                     bt_r[:, j, :], ct, start=True,
                                     stop=True)
                # x = (dot/sqrt(2))^2 = dot^2/2, whole group in one op
                nc.scalar.activation(out=x, in_=ps, func=AF.Square,
                                     scale=inv_sqrt2)
                x3 = x.rearrange("p (g k) -> p g k", g=G)
                # m = per-tile max, one grouped reduce
                nc.vector.reduce_max(m, x3, axis=AX.X)
                # nt0 = 1 - m   (nt = -t)
                nc.vector.tensor_scalar(out=nt, in0=m, scalar1=-1.0,
                                        scalar2=1.0, op0=OP.mult, op1=OP.add)
                # nt broadcast over k via a stride-0 middle AP dim
                nt_b = bass.AP(tensor=nt.tensor, offset=nt.offset,
                               ap=[nt.ap[0], nt.ap[1], [0, K]])
                return dict(base=base, x=x, x3=x3, y=y, o=o, nt=nt,
                            nt_b=nt_b, s12=s12, rs1=rs1, dlt=dlt)

            def emit_iter(s):
                x3, y, nt_b = s["x3"], s["y"], s["nt_b"]
                yL = y[:, :G * K]
                yL3 = yL.rearrange("p (g k) -> p g k", g=G)
                yR = y[:, G * K:]
                y6 = y.rearrange("p (g k) -> p g k", g=2 * G)
                s12, rs1, dlt, nt = s["s12"], s["rs1"], s["dlt"], s["nt"]
                s1 = s12[:, :G]
                s2 = s12[:, G:]
                # z = x + nt (group-wide broadcast add)       [VectorE]
                nc.vector.tensor_add(out=yL3, in0=x3, in1=nt_b)
                # y = relu(z) group-wide                      [ScalarE]
                nc.scalar.activation(out=yL, in_=yL, func=AF.Relu)
                # y^2 into the right half                     [ScalarE]
                nc.scalar.activation(out=yR, in_=yL, func=AF.Square)
                # s1|s2 in ONE grouped reduce                 [VectorE]
                nc.vector.reduce_sum(s12, y6, axis=AX.X)
                # nt -= (0.5*s2 - 0.5)/s1
                nc.vector.reciprocal(out=rs1, in_=s1)
                # dlt2 = (s2 - 1) * rs1  (= 2*delta)
                nc.vector.scalar_tensor_tensor(
                    out=dlt, in0=s2, scalar=1.0, in1=rs1,
                    op0=OP.subtract, op1=OP.mult)
                # nt = (dlt2 * -0.5) + nt
                nc.vector.scalar_tensor_tensor(
                    out=nt, in0=dlt, scalar=-0.5, in1=nt,
                    op0=OP.mult, op1=OP.add)

            def emit_final(s):
                x3, y, o, nt_b = s["x3"], s["y"], s["o"], s["nt_b"]
                yL = y[:, :G * K]
                yL3 = yL.rearrange("p (g k) -> p g k", g=G)
                # final: p = relu(x + nt)^2
                nc.vector.tensor_add(out=yL3, in0=x3, in1=nt_b)
                nc.scalar.activation(out=yL, in_=yL, func=AF.Relu)
                nc.scalar.activation(out=o, in_=yL, func=AF.Square)
                out_ap = out_d[s["base"]:s["base"] + GROUP_ROWS, :].rearrange(
                    "(p j) k -> p (j k)", j=G)
                nc.sync.dma_start(out=out_ap, in_=o)

            # emit groups in pairs, interleaving their dependency chains so
            # each engine's FIFO always has independent work
            assert ngroups % 2 == 0
            for gp in range(0, ngroups, 2):
                sA = emit_head(gp)
                sB = emit_head(gp + 1)
                for it in range(newton_iters):
                    emit_iter(sA)
                    emit_iter(sB)
                emit_final(sA)
                emit_final(sB)

    nc.compile()
    return nc


def _get_nc():
    key = (R_FULL, NEWTON_ITERS, SCALAR_RELU_TILES, FINAL_SCALAR_TILES)
    if key not in _CACHE:
        _CACHE[key] = _build(*key)
    return _CACHE[key]


def _entmax15_np(x):
    """Exact fp32 numpy port of the reference's sort-based entmax15
    along the last axis."""
    x = x.astype(np.float32) / np.float32(2.0)
    x = x - x.max(-1, keepdims=True)
    xs = np.flip(np.sort(x, axis=-1), axis=-1)
    k = x.shape[-1]
    rho = np.arange(1, k + 1, dtype=np.float32)
    mean = (np.cumsum(xs, -1, dtype=np.float32) / rho).astype(np.float32)
    mean_sq = (np.cumsum(xs * xs, -1, dtype=np.float32) / rho).astype(
        np.float32)
    ss = rho * (mean_sq - mean * mean)
    delta = (np.float32(1.0) - ss) / rho
    tau = mean - np.sqrt(np.clip(delta, 0.0, None))
    support = (tau <= xs).sum(-1, keepdims=True)
    tau_star = np.take_along_axis(tau, support - 1, axis=-1)
    return np.square(np.clip(x - tau_star, 0.0, None)).astype(np.float32)


def _losses_np(centers):
    c = centers.astype(np.float32)
    k = c.shape[0]
    p = _entmax15_np(c)
    m = (p @ p.T - np.eye(k, dtype=np.float32)).astype(np.float32)
    lossp = np.float32(np.linalg.norm((m @ m).astype(np.float32)) / k)
    eps = np.float32(1e-6)
    diffs = (c[:, None, :] - c[None, :, :] + eps).astype(np.float32)
    dist = np.sqrt((diffs * diffs).sum(-1, dtype=np.float32))
    upper = np.triu(np.ones((k, k), np.float32), k=1)
    total = np.float32(1e-10) + (dist * upper).sum(dtype=np.float32)
    n_pairs = k * (k - 1) // 2
    lossd = np.float32(1.0 / total / n_pairs)
    return lossp, lossd


def kernel(batch, cluster_centers):
    global LAST_RESULTS
    batch = np.ascontiguousarray(np.asarray(batch, dtype=np.float32))
    centers = np.ascontiguousarray(
        np.asarray(cluster_centers, dtype=np.float32))
    assert batch.shape == (B, N, D) and centers.shape == (K, D)

    # fold 1/sqrt(||c||) into the centers: (dot/sqrt(norm))^2 = dot^2/norm
    norm = np.sqrt((centers * centers).sum(-1, dtype=np.float32))
    cs = (centers / np.sqrt(norm)[:, None]).astype(np.float32)
    csT = np.ascontiguousarray(cs.T)  # [D, K]

    bpc = B // NCORES
    in_maps = []
    for i in range(NCORES):
        shard = batch[i * bpc:(i + 1) * bpc].reshape(R_FULL, D)
        in_maps.append({
            "batchT": np.ascontiguousarray(shard.T),
            "centersT": csT,
        })

    nc = _get_nc()
    res = run_bass_kernel_spmd(nc, in_maps, core_ids=list(range(NCORES)),
                               trace=TRACE, **TRACE_KWARGS)
    LAST_RESULTS = res

    out = np.concatenate([np.asarray(r["out"]) for r in res.results],
                         axis=0).reshape(B, N, K)
    lossp, lossd = _losses_np(centers)
    return out, lossp, lossd
